# revision 18
# baseline (speedup 1.0000x reference)
"""Causal multi-head attention (B=128, T=256, C=384, H=6, Dh=64) on 8 TRN2
NeuronCores, data-parallel over batch (16 batches per core, no collectives).

v3 design (vs. the v1 baseline at 346 us):
  - causal masking via gpsimd affine_select on the exp'd P tile (gpsimd is
    otherwise idle; its ucode affine_select measures ~255ns/[128,128]);
    no mask tensors, no mask multiplies on vector
  - P transposed on the PE (bf16 is_transpose, as v1; the DMA XBAR
    transpose measured 1.2us/block on hw - unusable)
  - AV merged to 2 matmuls per head (N=256 + N=128)
  - output projection is weight-stationary producing yT [C, T] in psum
    (host transposes back), 2 batches per psum bank
  - drains use nc.any so the tile scheduler load-balances scalar/vector
"""

import sys

sys.path.insert(0, "/opt/trn_rl_repo")

import numpy as np
import ml_dtypes

import concourse.bass as bass
import concourse.tile as tile
from concourse import mybir
from concourse.bass_utils import run_bass_kernel_spmd


def split_multi_waits(nc):
    """This walrus build accepts at most one sync-wait command per
    instruction; hoist extra waits into standalone InstEventSemaphore
    instructions on the same engine queue (queue waits run in order before
    the original instruction, so semantics are preserved)."""
    ctr = [0]

    def mk(engine, wait):
        ctr[0] += 1
        return mybir.InstEventSemaphore(
            name=f"WSPLIT-{ctr[0]}",
            engine=engine,
            ins=[],
            outs=[],
            sync_info=mybir.SyncInfo(on_wait=[wait], on_update=[]),
        )

    for f in nc.m.functions:
        for blk in f.blocks:
            insts = blk.instructions
            out = []
            for inst in insts:
                si = inst.sync_info
                if si is not None and len(si.on_wait) > 1:
                    waits = list(si.on_wait)
                    for w in waits[:-1]:
                        out.append(mk(inst.engine, w))
                    inst.sync_info = mybir.SyncInfo(
                        on_wait=[waits[-1]], on_update=list(si.on_update)
                    )
                out.append(inst)
            insts[:] = out
    return nc


N_CORES = 8
B, T, C = 128, 256, 384
H, DH = 6, 64
BL = B // N_CORES  # batches per core
GB = 4  # batches per group
NG = BL // GB
BF16 = mybir.dt.bfloat16
FP32 = mybir.dt.float32
AFT = mybir.ActivationFunctionType
ALU = mybir.AluOpType
SCALE = DH**-0.5  # 0.125


def build_kernel() -> bass.Bass:
    nc = bass.Bass()
    xT = nc.dram_tensor("xT", [BL, C, T], BF16, kind="ExternalInput")
    wqt = nc.dram_tensor("wqt", [C, C], BF16, kind="ExternalInput")  # Wq.T [C, D]
    wkt = nc.dram_tensor("wkt", [C, C], BF16, kind="ExternalInput")
    wvt = nc.dram_tensor("wvt", [C, C], BF16, kind="ExternalInput")
    wot = nc.dram_tensor("wot", [C, C], BF16, kind="ExternalInput")  # Wo.T [D, C]
    yT = nc.dram_tensor("yT", [BL, C, T], FP32, kind="ExternalOutput")

    with tile.TileContext(nc) as tc:
        with (
            tc.tile_pool(name="const", bufs=1) as const,
            tc.tile_pool(name="xp", bufs=2) as xp,
            tc.tile_pool(name="qkp", bufs=2) as qkp,
            tc.tile_pool(name="vp", bufs=6) as vp,
            tc.tile_pool(name="pp", bufs=6) as pp,
            tc.tile_pool(name="ptp", bufs=6) as ptp,
            tc.tile_pool(name="st", bufs=8) as st,
            tc.tile_pool(name="otp", bufs=2) as otp,
            tc.tile_pool(name="yp", bufs=3) as yp,
            tc.tile_pool(name="ps", bufs=1, space="PSUM") as ps,
        ):
            from concourse.masks import make_identity

            ident = const.tile([128, 128], BF16, tag="ident")
            make_identity(nc, ident)
            w_sb = {}
            for name, dram in (("wq", wqt), ("wk", wkt), ("wv", wvt), ("wo", wot)):
                w = const.tile([128, 3, C], BF16, tag=name)
                nc.sync.dma_start(out=w, in_=dram.rearrange("(k p) d -> p k d", p=128))
                w_sb[name] = w

            for g in range(NG):
                b0 = g * GB
                # ---- load xT for GB batches: [128, k, b, T] ----
                xt = xp.tile([128, 3, GB, T], BF16)
                for bi in range(GB):
                    nc.sync.dma_start(
                        out=xt[:, :, bi, :],
                        in_=xT[b0 + bi].rearrange("(k p) t -> p k t", p=128),
                    )

                # ---- QT/KT: [D, b, T]; N=512 matmuls (2 batches/bank) ----
                qt = qkp.tile([128, 3, GB, T], BF16, tag="qt")
                kt = qkp.tile([128, 3, GB, T], BF16, tag="kt")
                for dst, wname in ((qt, "wq"), (kt, "wk")):
                    w = w_sb[wname]
                    for d in range(3):
                        pb = ps.tile([128, GB, T], FP32, tag="qk", bufs=2)
                        for hf in range(2):
                            for k in range(3):
                                nc.tensor.matmul(
                                    pb[:, 2 * hf : 2 * hf + 2, :],
                                    lhsT=w[:, k, d * 128 : (d + 1) * 128],
                                    rhs=xt[:, k, 2 * hf : 2 * hf + 2, :],
                                    start=(k == 0),
                                    stop=(k == 2),
                                )
                        nc.scalar.copy(dst[:, d, :, :], pb)

                # ---- V = [T, D] per batch ----
                vs = []
                for bi in range(GB):
                    v = vp.tile([128, 2, C], BF16, tag="v")
                    for t2 in range(2):
                        psv = ps.tile([128, 512], FP32, tag="sc", bufs=2)
                        for k in range(3):
                            nc.tensor.matmul(
                                psv[:, 0:C],
                                lhsT=xt[:, k, bi, t2 * 128 : (t2 + 1) * 128],
                                rhs=w_sb["wv"][:, k, :],
                                start=(k == 0),
                                stop=(k == 2),
                            )
                        nc.scalar.copy(v[:, t2, :], psv[:, 0:C])
                    vs.append(v)

                # ---- attention; OT accumulated per group: [128, pair, b, T] ----
                otg = otp.tile([128, 3, GB, T], BF16)
                for bi in range(GB):
                    v = vs[bi]
                    for pair in range(3):
                        po = ps.tile([128, 512], FP32, tag="po", bufs=1)
                        tp = ps.tile(
                            [128, 2, 384], BF16, tag="tp", bufs=1, name="tp"
                        )
                        pt = ptp.tile([128, 2, 384], BF16, name="pt")
                        # both subs' P share one tile so mask/reduce/recip
                        # ops can cover both heads with strided APs
                        p2 = pp.tile([128, 2, 384], BF16, name="p2")
                        sums = st.tile([128, 4], FP32, name="sums")
                        rs = st.tile([128, 4], FP32, name="rs")
                        for sub in range(2):
                            h = 2 * pair + sub
                            doff = sub * 64
                            qh = qt[doff : doff + 64, pair, bi, :]
                            kh = kt[doff : doff + 64, pair, bi, :]
                            # scores: [tq0 x ks0:128 | tq1 x ks0:256]
                            sc = ps.tile(
                                [128, 512], FP32, tag="sc", bufs=2, name=f"sc{sub}"
                            )
                            nc.tensor.matmul(
                                sc[:, 0:128],
                                lhsT=qh[:, 0:128], rhs=kh[:, 0:128],
                                start=True, stop=True,
                            )
                            nc.tensor.matmul(
                                sc[:, 128:384],
                                lhsT=qh[:, 128:T], rhs=kh,
                                start=True, stop=True,
                            )
                            # exp; no max-subtraction (|scores*0.125| is O(5))
                            nc.scalar.activation(
                                p2[:, sub, :], sc[:, 0:384], AFT.Exp, scale=SCALE
                            )
                        # causal zeroing on idle gpsimd
                        for sub in range(2):
                            nc.gpsimd.affine_select(
                                out=p2[:, sub, 0:128], in_=p2[:, sub, 0:128],
                                compare_op=ALU.is_ge, fill=0.0,
                                base=0, pattern=[[-1, 128]], channel_multiplier=1,
                            )
                            nc.gpsimd.affine_select(
                                out=p2[:, sub, 256:384], in_=p2[:, sub, 256:384],
                                compare_op=ALU.is_ge, fill=0.0,
                                base=0, pattern=[[-1, 128]], channel_multiplier=1,
                            )
                            nc.vector.reduce_sum(
                                out=sums[:, sub : sub + 1], in_=p2[:, sub, 0:128],
                                axis=mybir.AxisListType.X,
                            )
                            nc.vector.reduce_sum(
                                out=sums[:, 2 + sub : 3 + sub],
                                in_=p2[:, sub, 128:384],
                                axis=mybir.AxisListType.X,
                            )
                        nc.vector.reciprocal(rs, sums)
                        for sub in range(2):
                            nc.vector.tensor_scalar_mul(
                                p2[:, sub, 0:128], p2[:, sub, 0:128],
                                rs[:, sub : sub + 1],
                            )
                            nc.vector.tensor_scalar_mul(
                                p2[:, sub, 128:384], p2[:, sub, 128:384],
                                rs[:, 2 + sub : 3 + sub],
                            )
                            # PT = [ks, tq]: PE transpose (bf16) into the
                            # pair-shared psum tile; one drain per pair
                            for blk in range(3):
                                c0 = blk * 128
                                nc.tensor.transpose(
                                    tp[:, sub, c0 : c0 + 128],
                                    p2[:, sub, c0 : c0 + 128],
                                    ident,
                                )
                        nc.vector.tensor_copy(pt, tp)
                        for sub in range(2):
                            h = 2 * pair + sub
                            doff = sub * 64
                            # AV: out OT[dh, t]; ts0 covers t0:256, ts1 adds
                            # t128:256
                            nc.tensor.matmul(
                                po[doff : doff + 64, 0:256],
                                lhsT=v[:, 0, h * 64 : (h + 1) * 64],
                                rhs=pt[:, sub, 0:256],
                                start=True, stop=False,
                                tile_position=(0, doff),
                                skip_group_check=True,
                            )
                            nc.tensor.matmul(
                                po[doff : doff + 64, 128:256],
                                lhsT=v[:, 1, h * 64 : (h + 1) * 64],
                                rhs=pt[:, sub, 256:384],
                                start=False, stop=True,
                                tile_position=(0, doff),
                                skip_group_check=True,
                            )
                        nc.scalar.copy(otg[:, pair, bi, :], po[:, 0:256])

                # ---- yT = (WoT stationary) @ OT : [C, b, T] ----
                for cd in range(3):
                    pb = ps.tile([128, GB, T], FP32, tag="qk", bufs=2, name="oproj")
                    for hf in range(2):
                        for k in range(3):
                            nc.tensor.matmul(
                                pb[:, 2 * hf : 2 * hf + 2, :],
                                lhsT=w_sb["wo"][:, k, cd * 128 : (cd + 1) * 128],
                                rhs=otg[:, k, 2 * hf : 2 * hf + 2, :],
                                start=(k == 0),
                                stop=(k == 2),
                            )
                    ys = yp.tile([128, GB, T], FP32)
                    nc.vector.tensor_copy(ys, pb)
                    nc.sync.dma_start(
                        out=yT[
                            b0 : b0 + GB, cd * 128 : (cd + 1) * 128, :
                        ].rearrange("b p t -> p b t"),
                        in_=ys,
                    )
    return nc


_NC = None


def _get_nc():
    global _NC
    if _NC is None:
        _NC = split_multi_waits(build_kernel())
    return _NC


def kernel(x, Wq, Wk, Wv, Wo, _trace=False):
    bf16 = ml_dtypes.bfloat16
    wq_t = np.ascontiguousarray(Wq.T).astype(bf16)
    wk_t = np.ascontiguousarray(Wk.T).astype(bf16)
    wv_t = np.ascontiguousarray(Wv.T).astype(bf16)
    wo_t = np.ascontiguousarray(Wo.T).astype(bf16)
    in_maps = []
    for i in range(N_CORES):
        xs = x[i * BL : (i + 1) * BL]  # [BL, T, C]
        xs_t = np.ascontiguousarray(xs.transpose(0, 2, 1)).astype(bf16)
        in_maps.append(
            {"xT": xs_t, "wqt": wq_t, "wkt": wk_t, "wvt": wv_t, "wot": wo_t}
        )
    res = run_bass_kernel_spmd(
        _get_nc(), in_maps, list(range(N_CORES)), trace=_trace
    )
    out = np.concatenate(
        [r["yT"].transpose(0, 2, 1) for r in res.results], axis=0
    )
    out = np.ascontiguousarray(out).astype(np.float32)
    if _trace:
        return out, res
    return out
